# revision 74
# baseline (speedup 1.0000x reference)
"""Trainium2 Bass kernel for AdaptivePositionAwareAttention.

Banded softmax + closed-form far field: pos_w = exp(-|i-j|/2) kills
exp(f)-1 beyond ~90 positions, so per 128-row query block t only key
blocks t-1..t+1 (WIN=384) matter:

    y_i = zr_i * (VsumWo + sum_band (e^{f_ij}-1) VW_j) + (bv@Wo + bo)

where VW = xp @ (Wo Wv)^T, VsumWo = colsum of VW over the full sequence,
and zr_i = 1/(S - WIN + sum_band e^f).  The task/content/fusion branches
reduce to a per-row scalar g with f = base * g.

v2 layout: the host pre-adds pos, pre-transposes x to feature-major fp8
(xpT8), pre-computes the far-field row (vswo = colsum(xp) @ WvoT), the
per-batch task scalar, and A = Wq^T Wk so no K projection is needed:
scores = (xp@A) @ xp^T, both matmuls fp8 DoubleRow.  ci / fw use
free-stationary column matmuls ([128,1] outputs); sigmoids run as
exp + reciprocal so the Activation engine never reloads its table;
E'^T is produced by DMA-XBAR transposes on the SP queue.

Sharding: 8 cores = (batch 0..3) x (sequence half); each core owns 1024
query rows plus one 128-row zero-padded halo block per side.  Sequence
edges ride on per-core masked pos_w data, so all cores run one SPMD
graph.
"""

import math

import numpy as np
import ml_dtypes

import concourse.bass as bass
from concourse import bacc, mybir, tile

# tensor_tensor_reduce and mixed DR/non-DR psum groups crash real HW
# (NRT_EXEC_UNIT_UNRECOVERABLE) though CoreSim accepts them — keep off.
USE_TTR = False
USE_H2DR = False

B, S, H = 4, 2048, 768
HC = H // 128             # 6 feature chunks
NB = 10                   # blocks per core incl. halo (1280 rows)
WIN = 384                 # key window = 3 blocks
F32 = mybir.dt.float32
BF16 = mybir.dt.bfloat16
F8 = mybir.dt.float8e4
AF = mybir.ActivationFunctionType
ALU = mybir.AluOpType
AX = mybir.AxisListType
DR = mybir.MatmulPerfMode.DoubleRow

A_SC = 128.0              # scale on A = Wq^T Wk to dodge fp8 subnormals
QSC = 16.0                # stored-QA scale
WVO_SC = 16.0             # scale on (Wo Wv)^T
WC1_SC = 64.0
WC2_SC = 64.0
WC3_SC = 64.0

# smf (f32 [128, 21]) column map
SMF_BC1 = 0    # 3 cols
SMF_BC2 = 3    # 2 cols
SMF_BF1 = 5    # 6 cols
SMF_VQ = 11    # 6 cols: (bq @ Wk) * QSC
SMF_TS1 = 17   # 1 - 0.5*ts
SMF_TS = 18    # ts
SMF_NBC3 = 19  # -bc3 (bcast)
SMF_NBF2 = 20  # -bf2 (bcast)
SMF_ID = 21    # 128 cols: f32 identity (for f32 PE transposes)
SMF_NEG1 = SMF_ID + 128   # column of -1.0
SMF_N = SMF_NEG1 + 1

# bfp (bf16) column map
BFP_PW = 0                    # 3*WIN pw tables (first/mid/last)
BFP_WF2 = 3 * WIN             # 6 cols
NBF = BFP_WF2 + 6

_cache = {}


def r3(ap, c):
    return ap.rearrange("p (c s) -> p c s", c=c)


def build_kernel(dbg=False):
    nc = bacc.Bacc(None, target_bir_lowering=False)

    def din(name, shape, dt=F32):
        return nc.dram_tensor(name, shape, dt, kind="ExternalInput")

    xpT8d = din("xpT8", [128, HC * NB * 128], F8)  # feature-major (x+pos)
    a8d = din("a8", [H, H], F8)                    # Wq^T Wk * A_SC  [f, g]
    wvo8d = din("wvo8", [H, H], F8)                # (Wo Wv)^T * WVO_SC
    wc18d = din("wc18", [H, 384], F8)              # Wc1^T * WC1_SC
    wc28d = din("wc28", [512, 256], F8)            # Wc2^T * WC2_SC, pad 192->256
    f8pd = din("f8p", [128, 2], F8)                # Wc3 * WC3_SC columns
    wf1d = din("wf1p", [3, H], BF16)               # Wf1^T / S
    smfd = din("smf", [128, SMF_N], F32)
    bfpd = din("bfp", [128, NBF], BF16)            # ident | pw f/m/l | wf2

    y = nc.dram_tensor("y", [1024, H], BF16, kind="ExternalOutput")
    zrd = nc.dram_tensor("zr8", [128, 8], F32, kind="ExternalOutput")

    with tile.TileContext(nc) as tc:
        with (
            tc.tile_pool(name="pers", bufs=1) as pers,
            tc.tile_pool(name="pQ", bufs=2, space="PSUM") as pQ,
            tc.tile_pool(name="pB", bufs=4, space="PSUM") as pB,
            tc.tile_pool(name="pC", bufs=2, space="PSUM") as pC,
        ):
            # ---------- startup PE warmup (no data dependencies) ----------
            scr0 = pers.tile([128, 128], F32, tag="scr0")
            nc.gpsimd.memset(scr0[:], 0)
            for _ in range(12):
                pd = pC.tile([128, 128], F32, tag="pc")
                nc.tensor.transpose(pd[:], scr0[:], scr0[:])

            # ---------- DMA loads (wave 0: what QA needs first) ----------
            a8 = pers.tile([128, HC * H], F8, tag="a8")
            ha = HC * H // 2
            nc.sync.dma_start(
                r3(a8[:, 0:ha], 3),
                a8d[0:H // 2, :].rearrange("(c p) n -> p c n", c=3))
            nc.scalar.dma_start(
                r3(a8[:, ha:2 * ha], 3),
                a8d[H // 2:H, :].rearrange("(c p) n -> p c n", c=3))
            xpT8 = pers.tile([128, HC * NB * 128], F8, tag="xpT8")
            h3 = HC * NB * 128 // 3
            smf_t = pers.tile([128, SMF_N], F32, tag="smf")
            nc.gpsimd.dma_start(smf_t[:], smfd[:])
            nc.sync.dma_start(xpT8[:, 0:h3], xpT8d[:, 0:h3])
            nc.sync.dma_start(xpT8[:, h3:2 * h3], xpT8d[:, h3:2 * h3])
            nc.gpsimd.dma_start(xpT8[:, 2 * h3:3 * h3], xpT8d[:, 2 * h3:3 * h3])
            bfp_t = pers.tile([128, NBF], BF16, tag="bfp")
            nc.gpsimd.dma_start(bfp_t[:], bfpd[:])
            identb32 = smf_t[:, SMF_ID:SMF_ID + 128]
            pw_at = {1: BFP_PW, 8: BFP_PW + 2 * WIN}  # t -> col (else mid)
            wf2_b = bfp_t[:, BFP_WF2:BFP_WF2 + 6]
            # wave 1
            wvo8 = pers.tile([128, HC * H], F8, tag="wvo8")
            nc.sync.dma_start(
                r3(wvo8[:], HC), wvo8d[:].rearrange("(c p) n -> p c n", c=HC))
            wc18 = pers.tile([128, HC * 384], F8, tag="wc18")
            nc.sync.dma_start(
                r3(wc18[:], HC), wc18d[:].rearrange("(c p) n -> p c n", c=HC))
            wc28 = pers.tile([128, 4 * 256], F8, tag="wc28")
            nc.gpsimd.dma_start(
                r3(wc28[:], 4), wc28d[:].rearrange("(c p) n -> p c n", c=4))
            f8p_t = pers.tile([128, 2], F8, tag="f8p")
            nc.gpsimd.dma_start(f8p_t[:], f8pd[:])
            wf1_t = pers.tile([3, H], BF16, tag="wf1")
            nc.gpsimd.dma_start(wf1_t[:], wf1d[:])

            xp8 = r3(xpT8[:], HC)
            a8v = r3(a8[:], HC)
            wvov = r3(wvo8[:], HC)
            wc1v = r3(wc18[:], HC)
            wc2v = r3(wc28[:], 4)

            # ---------- persistent activations ----------
            qa8 = pers.tile([128, HC * 1024], F8, tag="qa8")
            h1T = pers.tile([128, 4 * 1024], F8, tag="h1T")
            h2T = pers.tile([128, 2 * 1024], F8, tag="h2T")
            fu1s = pers.tile([128, HC * 1024], BF16, tag="fu1s")
            vw = pers.tile([128, NB * H], BF16, tag="vw")
            bp8 = pers.tile([128, 8 * WIN], BF16, tag="bp8")
            rs8 = pers.tile([128, 8], F32, tag="rs8")
            ci8 = pers.tile([128, 8], F32, tag="ci8")
            fw8 = pers.tile([128, 8], F32, tag="fw8")
            g8 = pers.tile([128, 8], F32, tag="g8")
            eci = pers.tile([128, 8], F32, tag="eci")
            efw = pers.tile([128, 8], F32, tag="efw")
            pre8 = pers.tile([128, 8], F32, tag="pre8")
            zr8 = pers.tile([128, 8], F32, tag="zr8")
            ftin = pers.tile([128, 24], F32, tag="ftin")
            fin = pers.tile([3, 1024], BF16, tag="fin")
            qav = r3(qa8[:], HC)
            h1v = r3(h1T[:], 4)
            h2v = r3(h2T[:], 2)
            fuv = r3(fu1s[:], HC)
            vwv = r3(vw[:], NB)

            # ---------- phase helpers ----------
            def qa_wave(j2):
                for oc in range(HC):
                    p = pQ.tile([128, 512], F32, tag="pq")
                    for c in range(3):
                        nc.tensor.matmul(
                            p[:],
                            a8v[:, 2 * c:2 * c + 2, oc * 128:(oc + 1) * 128],
                            xp8[:, 2 * c:2 * c + 2,
                                128 + j2 * 512:128 + (j2 + 1) * 512],
                            start=(c == 0), stop=(c == 2),
                            perf_mode=DR, skip_group_check=True)
                    dst = qa8[:, oc * 1024 + j2 * 512:
                              oc * 1024 + (j2 + 1) * 512]
                    vq = smf_t[:, SMF_VQ + oc:SMF_VQ + oc + 1]
                    if oc % 2 == 0:
                        nc.vector.tensor_scalar(dst, p[:], QSC / A_SC, vq,
                                                ALU.mult, ALU.add)
                    else:
                        nc.scalar.activation(dst, p[:], AF.Identity, bias=vq,
                                             scale=QSC / A_SC)

            def score_block(t):
                p = pB.tile([128, WIN], F32, tag="pb")
                for n0, nn in ((0, 256), (256, 128)):
                    for c in range(3):
                        nc.tensor.matmul(
                            p[:, n0:n0 + nn],
                            qav[:, 2 * c:2 * c + 2, (t - 1) * 128:t * 128],
                            xp8[:, 2 * c:2 * c + 2,
                                (t - 1) * 128 + n0:(t - 1) * 128 + n0 + nn],
                            start=(c == 0), stop=(c == 2),
                            perf_mode=DR, skip_group_check=True)
                pwc = pw_at.get(t, BFP_PW + WIN)
                nc.vector.tensor_tensor(bp8[:, (t - 1) * WIN:t * WIN],
                                        p[:], bfp_t[:, pwc:pwc + WIN],
                                        ALU.mult)
                nc.vector.tensor_reduce(rs8[:, t - 1:t],
                                        bp8[:, (t - 1) * WIN:t * WIN],
                                        AX.X, ALU.add)

            def h1_wave(j2):
                for oc in range(3):
                    p = pQ.tile([128, 512], F32, tag="pq")
                    for c in range(3):
                        nc.tensor.matmul(
                            p[:],
                            wc1v[:, 2 * c:2 * c + 2, oc * 128:(oc + 1) * 128],
                            xp8[:, 2 * c:2 * c + 2,
                                128 + j2 * 512:128 + (j2 + 1) * 512],
                            start=(c == 0), stop=(c == 2),
                            perf_mode=DR, skip_group_check=True)
                    nc.scalar.activation(
                        h1T[:, oc * 1024 + j2 * 512:
                            oc * 1024 + (j2 + 1) * 512],
                        p[:], AF.Relu, scale=1.0 / WC1_SC,
                        bias=smf_t[:, SMF_BC1 + oc:SMF_BC1 + oc + 1])

            def h2_wave(j2):
                for oc in range(2):
                    p = pQ.tile([128, 512], F32, tag="pq")
                    for c2 in range(2):
                        nc.tensor.matmul(
                            p[:],
                            wc2v[:, 2 * c2:2 * c2 + 2, oc * 128:(oc + 1) * 128],
                            h1v[:, 2 * c2:2 * c2 + 2,
                                j2 * 512:(j2 + 1) * 512],
                            start=(c2 == 0), stop=(c2 == 1),
                            perf_mode=DR, skip_group_check=True)
                    nc.scalar.activation(
                        h2T[:, oc * 1024 + j2 * 512:
                            oc * 1024 + (j2 + 1) * 512],
                        p[:], AF.Relu, scale=1.0 / WC2_SC,
                        bias=smf_t[:, SMF_BC2 + oc:SMF_BC2 + oc + 1])

            ci_ps = pC.tile([128, 128], F32, tag="pc")

            def ci_cols(ts_):
                for t in ts_:
                    for cc in range(2):
                        nc.tensor.matmul(
                            ci_ps[:, t:t + 1],
                            h2v[:, cc, t * 128:(t + 1) * 128],
                            f8p_t[:, cc:cc + 1],
                            start=(cc == 0), stop=(cc == 1),
                            skip_group_check=True)

            def vw_block(u, dve_only=False):
                for n2 in range(2):
                    p = pB.tile([128, WIN], F32, tag="pb")
                    for n0, nn in ((0, 256), (256, 128)):
                        for c in range(3):
                            nc.tensor.matmul(
                                p[:, n0:n0 + nn],
                                xp8[:, 2 * c:2 * c + 2, u * 128:(u + 1) * 128],
                                wvov[:, 2 * c:2 * c + 2,
                                     n2 * WIN + n0:n2 * WIN + n0 + nn],
                                start=(c == 0), stop=(c == 2),
                                perf_mode=DR, skip_group_check=True)
                    dst = vw[:, u * H + n2 * WIN:u * H + (n2 + 1) * WIN]
                    if dve_only or (u * 2 + n2) % 2 == 0:
                        nc.vector.tensor_scalar_mul(dst, p[:], 1.0 / WVO_SC)
                    else:
                        nc.scalar.activation(dst, p[:], AF.Identity,
                                             scale=1.0 / WVO_SC)

            ftv = ftin[:].rearrange("p (t k) -> p t k", t=8)

            def fusion_in():
                nc.scalar.activation(eci[:], ci_ps[:, 0:8], AF.Exp,
                                     scale=-1.0 / WC3_SC,
                                     bias=smf_t[:, SMF_NBC3:SMF_NBC3 + 1])
                nc.vector.tensor_scalar_add(eci[:], eci[:], 1.0)
                nc.vector.reciprocal(ci8[:], eci[:])
                nc.vector.tensor_copy(ftv[:, :, 0], rs8[:])
                nc.vector.tensor_scalar_mul(ftv[:, :, 1], rs8[:],
                                            smf_t[:, SMF_TS:SMF_TS + 1])
                nc.vector.tensor_tensor(ftv[:, :, 2], rs8[:], ci8[:],
                                        ALU.mult)
                nc.vector.tensor_scalar(pre8[:], ci8[:], -0.5,
                                        smf_t[:, SMF_TS1:SMF_TS1 + 1],
                                        ALU.mult, ALU.add)
                for t in range(8):
                    pf = pC.tile([128, 128], F32, tag="pc")
                    nc.tensor.transpose(pf[0:3, :], ftin[:, 3 * t:3 * t + 3],
                                        identb32)
                    if t % 2 == 0:
                        nc.vector.tensor_copy(fin[:, t * 128:(t + 1) * 128],
                                              pf[0:3, :])
                    else:
                        nc.scalar.activation(fin[:, t * 128:(t + 1) * 128],
                                             pf[0:3, :], AF.Identity)

            def fu1_oc(oc):
                for hh in range(2):
                    p = pQ.tile([128, 512], F32, tag="pq")
                    nc.tensor.matmul(
                        p[:], wf1_t[0:3, oc * 128:(oc + 1) * 128],
                        fin[:, hh * 512:(hh + 1) * 512],
                        start=True, stop=True, skip_group_check=True)
                    dst = fu1s[:, oc * 1024 + hh * 512:
                               oc * 1024 + (hh + 1) * 512]
                    bcol = smf_t[:, SMF_BF1 + oc:SMF_BF1 + oc + 1]
                    if (oc + hh) % 2 == 0:
                        nc.vector.tensor_scalar(dst, p[:], bcol, 0.0,
                                                ALU.add, ALU.max)
                    else:
                        nc.scalar.activation(dst, p[:], AF.Relu,
                                             bias=bcol)

            def fusion_out():
                fw_ps = pC.tile([128, 128], F32, tag="pc")
                for t in range(8):
                    for oc in range(HC):
                        nc.tensor.matmul(
                            fw_ps[:, t:t + 1],
                            fuv[:, oc, t * 128:(t + 1) * 128],
                            wf2_b[:, oc:oc + 1],
                            start=(oc == 0), stop=(oc == HC - 1),
                            skip_group_check=True)
                nc.scalar.activation(efw[:], fw_ps[:, 0:8], AF.Exp,
                                     scale=-1.0,
                                     bias=smf_t[:, SMF_NBF2:SMF_NBF2 + 1])
                nc.vector.tensor_scalar_add(efw[:], efw[:], 1.0)
                nc.vector.reciprocal(fw8[:], efw[:])
                nc.vector.tensor_tensor(g8[:], fw8[:], pre8[:], ALU.mult)
                nc.vector.tensor_scalar(g8[:], g8[:], -1.0, 1.0,
                                        ALU.mult, ALU.add)

            def pe_warm(n):
                # dependency-free matmuls that keep the PE p-state ramped
                # while the first tail head flows through Act/DVE/SP
                for _ in range(n):
                    pd = pC.tile([128, 128], F32, tag="pc")
                    nc.tensor.transpose(pd[:], smf_t[:, SMF_ID:SMF_ID + 128],
                                        identb32)

            # ---------- tail ops ----------
            with (
                tc.tile_pool(name="eb", bufs=8) as eb,
                tc.tile_pool(name="epb", bufs=8) as epb,
                tc.tile_pool(name="etp", bufs=8) as etp,
                tc.tile_pool(name="zrp", bufs=24) as zrp,
                tc.tile_pool(name="ytp", bufs=8) as ytp,
            ):
                heads = {}

                def tail_head(t):
                    e_t = eb.tile([128, WIN], BF16, tag="et")
                    zc = zrp.tile([128, 1], F32, tag="zc")
                    nc.scalar.activation(e_t[:], bp8[:, (t - 1) * WIN:t * WIN],
                                         AF.Exp, scale=g8[:, t - 1:t],
                                         accum_out=zc[:])
                    ep = epb.tile([128, WIN], BF16, tag="ep")
                    nc.scalar.activation(ep[:], e_t[:], AF.Identity,
                                         bias=smf_t[:, SMF_NEG1:SMF_NEG1 + 1])
                    zd = zrp.tile([128, 1], F32, tag="zd")
                    nc.vector.tensor_scalar_add(zd[:], zc[:], float(S - WIN))
                    nc.vector.reciprocal(zr8[:, t - 1:t], zd[:])
                    eT = etp.tile([128, 3 * 128], BF16, tag="eT")
                    eTv = r3(eT[:], 3)
                    nc.sync.dma_start(eTv[:], ep[:], transpose=True)
                    heads[t] = eTv

                def tail_y(t, n2_act=True):
                    eTv = heads.pop(t)
                    zrc = zr8[:, t - 1:t]
                    y_t = ytp.tile([128, H], BF16, tag="yt")
                    for n2 in range(2):
                        p = pB.tile([128, WIN], F32, tag="pb")
                        for w in range(3):
                            nc.tensor.matmul(
                                p[:], eTv[:, w, :],
                                vwv[:, t - 1 + w, n2 * WIN:(n2 + 1) * WIN],
                                start=(w == 0), stop=(w == 2),
                                skip_group_check=True)
                        dst = y_t[:, n2 * WIN:(n2 + 1) * WIN]
                        if n2 == 1 and n2_act:
                            nc.scalar.activation(dst, p[:], AF.Identity,
                                                 scale=zrc)
                        else:
                            nc.vector.tensor_scalar_mul(dst, p[:], zrc)
                    nc.gpsimd.dma_start(y[(t - 1) * 128:t * 128, :], y_t[:])

                # ---------- emission schedule ----------
                qa_wave(0)
                qa_wave(1)
                for t in range(1, 9):
                    score_block(t)
                h1_wave(0)
                h1_wave(1)
                h2_wave(0)
                h2_wave(1)
                ci_cols(range(0, 8))
                for u in range(0, NB):
                    vw_block(u)
                fusion_in()
                for oc in range(HC):
                    fu1_oc(oc)
                fusion_out()
                tail_head(1)
                tail_head(2)
                tail_head(3)
                tail_head(4)
                pe_warm(36)
                tail_y(1, n2_act=False)
                tail_head(5)
                tail_y(2, n2_act=False)
                tail_head(6)
                tail_y(3, n2_act=False)
                tail_head(7)
                tail_y(4, n2_act=False)
                tail_head(8)
                nc.sync.dma_start(zrd[:], zr8[:])
                for t in range(5, 9):
                    tail_y(t, n2_act=False)

    nc.compile()
    return nc


def prep_inputs(x, task_id, pos_emb, Wq, bq, Wk, bk, Wv, bv, Wo, bo,
                task_table, Wt1, bt1, Wt2, bt2,
                Wc1, bc1, Wc2, bc2, Wc3, bc3,
                Wf1, bf1, Wf2, bf2):
    f = np.float32
    bf = ml_dtypes.bfloat16
    f8 = mybir.dt.np(F8)
    T = lambda a: np.ascontiguousarray(np.asarray(a, dtype=f).T)
    c8 = lambda a: np.clip(np.asarray(a, dtype=f), -240, 240).astype(f8)
    sig = lambda a: 1.0 / (1.0 + np.exp(-a))

    Wq_, Wk_, Wv_, Wo_ = (np.asarray(w, f) for w in (Wq, Wk, Wv, Wo))
    wvo = T(Wv_) @ T(Wo_)                    # (Wo Wv)^T
    A = Wq_.T @ Wk_                          # [f, g]
    vq = (np.asarray(bq, f) @ Wk_) * QSC     # [768]

    smf = np.zeros((128, SMF_N), f)
    smf[:, SMF_ID:SMF_ID + 128] = np.eye(128, dtype=f)
    smf[:, SMF_NEG1] = -1.0
    smf[:, SMF_BC1:SMF_BC1 + 3] = np.asarray(bc1, f).reshape(3, 128).T
    bc2p = np.zeros(256, f); bc2p[:192] = np.asarray(bc2, f)
    smf[:, SMF_BC2:SMF_BC2 + 2] = bc2p.reshape(2, 128).T
    smf[:, SMF_BF1:SMF_BF1 + 6] = np.asarray(bf1, f).reshape(HC, 128).T
    smf[:, SMF_VQ:SMF_VQ + 6] = vq.reshape(HC, 128).T
    smf[:, SMF_NBC3] = -float(np.asarray(bc3, f).reshape(-1)[0])
    smf[:, SMF_NBF2] = -float(np.asarray(bf2, f).reshape(-1)[0])

    # pw tables: pos_w / (sqrt(H) * QSC); first/last mask sequence edges
    p_ = np.arange(128)[:, None]; c_ = np.arange(WIN)[None, :]
    pwm = (np.exp(-np.abs(128 + p_ - c_) / 2.0) / (math.sqrt(H) * QSC)).astype(f)

    wc2p = np.zeros((512, 256), f); wc2p[:384, :192] = T(Wc2)
    wc3p = np.zeros((256,), f); wc3p[:192] = np.asarray(Wc3, f).reshape(-1)
    f8pk = (wc3p.reshape(2, 128).T) * WC3_SC

    wf1p = (T(Wf1) / S).astype(bf)                       # [3, 768]
    bvoo = (np.asarray(bv, f) @ Wo_.T + np.asarray(bo, f)).reshape(H)

    # per-batch task scalar
    task_table = np.asarray(task_table, f); task_id = np.asarray(task_id)
    te = task_table[task_id.astype(np.int64)]            # [B, 64]
    t1 = np.maximum(te @ np.asarray(Wt1, f).T + np.asarray(bt1, f), 0.0)
    tw = sig(t1 @ np.asarray(Wt2, f).T + np.asarray(bt2, f))
    ts_all = tw.mean(-1)                                 # [B]

    shared = {
        "a8": c8(A * A_SC), "wvo8": c8(wvo * WVO_SC),
        "wc18": c8(T(Wc1) * WC1_SC), "wc28": c8(wc2p * WC2_SC),
        "f8p": c8(f8pk), "wf1p": wf1p,
    }

    x = np.asarray(x, f); pos_emb = np.asarray(pos_emb, f)
    in_maps = []
    vswo_all = []
    for core in range(8):
        b, half = core // 2, core % 2
        xp_full = x[b] + pos_emb[0]                      # [S, H]
        g0 = 1024 * half - 128
        xp = np.zeros((NB * 128, H), f)
        lo, hi = max(0, g0), min(S, g0 + NB * 128)
        xp[lo - g0:hi - g0] = xp_full[lo:hi]
        # feature-major fp8: [128, HC, NB*128]
        xpT = xp.T.reshape(HC, 128, NB * 128).transpose(1, 0, 2)
        xpT8 = c8(xpT.reshape(128, HC * NB * 128))

        pwf = pwm.copy(); pwl = pwm.copy()
        if half == 0:
            pwf[:, :128] = 0
        if half == 1:
            pwl[:, 256:] = 0
        bfpk = np.zeros((128, NBF), bf)
        bfpk[:, BFP_PW:BFP_PW + WIN] = pwf.astype(bf)
        bfpk[:, BFP_PW + WIN:BFP_PW + 2 * WIN] = pwm.astype(bf)
        bfpk[:, BFP_PW + 2 * WIN:BFP_PW + 3 * WIN] = pwl.astype(bf)
        bfpk[:, BFP_WF2:BFP_WF2 + 6] = \
            np.asarray(Wf2, f).reshape(HC, 128).T.astype(bf)

        smfc = smf.copy()
        ts = float(ts_all[b])
        smfc[:, SMF_TS1] = 1.0 - 0.5 * ts
        smfc[:, SMF_TS] = ts

        m = dict(shared)
        m.update({"xpT8": xpT8, "bfp": bfpk, "smf": smfc})
        in_maps.append(m)
        vswo_all.append((xp_full.sum(0) @ wvo).astype(f))
    return in_maps, bvoo, vswo_all


class _Runner:
    """Compile the SPMD graph once and keep a reusable jitted callable."""

    def __init__(self, nc, n_cores=8):
        import jax
        from jax.sharding import Mesh, PartitionSpec
        from jax.experimental.shard_map import shard_map
        from concourse import bass2jax, mybir as _mb
        bass2jax.install_neuronx_cc_hook()
        self.nc = nc
        partition_name = (nc.partition_id_tensor.name
                          if nc.partition_id_tensor else None)
        in_names, out_names, out_avals, zero_outs = [], [], [], []
        for alloc in nc.m.functions[0].allocations:
            if not isinstance(alloc, _mb.MemoryLocationSet):
                continue
            name = alloc.memorylocations[0].name
            if alloc.kind == "ExternalInput":
                if name != partition_name:
                    in_names.append(name)
            elif alloc.kind == "ExternalOutput":
                shape = tuple(alloc.tensor_shape)
                dtype = _mb.dt.np(alloc.dtype)
                out_names.append(name)
                out_avals.append(jax.core.ShapedArray(shape, dtype))
                zero_outs.append(np.zeros(shape, dtype))
        self.in_names = list(in_names)
        self.out_names = out_names
        self.out_avals = out_avals
        self.zero_outs = zero_outs
        self.n_cores = n_cores
        n_params = len(self.in_names)
        all_in = list(self.in_names) + list(out_names)
        if partition_name is not None:
            all_in.append(partition_name)

        def _body(*args):
            operands = list(args)
            if partition_name is not None:
                operands.append(bass2jax.partition_id_tensor())
            outs = bass2jax._bass_exec_p.bind(
                *operands,
                out_avals=tuple(out_avals),
                in_names=tuple(all_in),
                out_names=tuple(out_names),
                lowering_input_output_aliases=(),
                sim_require_finite=True,
                sim_require_nnan=True,
                nc=nc,
            )
            return tuple(outs)

        devices = jax.devices()[:n_cores]
        mesh = Mesh(np.asarray(devices), ("core",))
        n_outs = len(out_names)
        in_specs = (PartitionSpec("core"),) * (n_params + n_outs)
        out_specs = (PartitionSpec("core"),) * n_outs
        self.fn = jax.jit(
            shard_map(_body, mesh=mesh, in_specs=in_specs,
                      out_specs=out_specs, check_rep=False),
            keep_unused=True)

    def concat_inputs(self, in_maps):
        return [np.concatenate([np.asarray(in_maps[c][k])
                                for c in range(self.n_cores)], axis=0)
                for k in self.in_names]

    def zeros(self):
        return [np.zeros((self.n_cores * z.shape[0],) + z.shape[1:], z.dtype)
                for z in self.zero_outs]

    def __call__(self, concat_in, zeros=None):
        import jax
        if zeros is None:
            zeros = self.zeros()
        outs = jax.block_until_ready(self.fn(*concat_in, *zeros))
        return outs


def get_runner(dbg=False):
    if "runner" not in _cache:
        _cache["runner"] = _Runner(build_kernel())
    return _cache["runner"]


def kernel(**inputs):
    rn = get_runner()
    in_maps, bvoo, vswo_all = prep_inputs(**inputs)
    outs = rn(rn.concat_inputs(in_maps))
    yc = np.asarray(outs[rn.out_names.index("y")]).reshape(8, 1024, H)
    zc = np.asarray(outs[rn.out_names.index("zr8")]).reshape(8, 128, 8)
    out = np.zeros((B, S, H), np.float32)
    for core in range(8):
        b, half = core // 2, core % 2
        zr = zc[core].T.reshape(1024, 1)
        out[b, 1024 * half:1024 * (half + 1)] = \
            yc[core].astype(np.float32) + zr * vswo_all[core] + bvoo
    return out


# revision 75
# speedup vs baseline: 1.2041x; 1.2041x over previous
"""Trainium2 Bass kernel for AdaptivePositionAwareAttention.

Banded softmax + closed-form far field: pos_w = exp(-|i-j|/2) kills
exp(f)-1 beyond ~90 positions, so per 128-row query block t only key
blocks t-1..t+1 (WIN=384) matter:

    y_i = zr_i * (VsumWo + sum_band (e^{f_ij}-1) VW_j) + (bv@Wo + bo)

where VW = xp @ (Wo Wv)^T, VsumWo = colsum of VW over the full sequence,
and zr_i = 1/(S - WIN + sum_band e^f).  The task/content/fusion branches
reduce to a per-row scalar g with f = base * g.

v2 layout: the host pre-adds pos, pre-transposes x to feature-major fp8
(xpT8), pre-computes the far-field row (vswo = colsum(xp) @ WvoT), the
per-batch task scalar, and A = Wq^T Wk so no K projection is needed:
scores = (xp@A) @ xp^T, both matmuls fp8 DoubleRow.  ci / fw use
free-stationary column matmuls ([128,1] outputs); sigmoids run as
exp + reciprocal so the Activation engine never reloads its table;
E'^T is produced by DMA-XBAR transposes on the SP queue.

Sharding: 8 cores = (batch 0..3) x (sequence half); each core owns 1024
query rows plus one 128-row zero-padded halo block per side.  Sequence
edges ride on per-core masked pos_w data, so all cores run one SPMD
graph.
"""

import math

import numpy as np
import ml_dtypes

import concourse.bass as bass
from concourse import bacc, mybir, tile

# tensor_tensor_reduce and mixed DR/non-DR psum groups crash real HW
# (NRT_EXEC_UNIT_UNRECOVERABLE) though CoreSim accepts them — keep off.
USE_TTR = False
USE_H2DR = False

B, S, H = 4, 2048, 768
HC = H // 128             # 6 feature chunks
NB = 10                   # blocks per core incl. halo (1280 rows)
WIN = 384                 # key window = 3 blocks
F32 = mybir.dt.float32
BF16 = mybir.dt.bfloat16
F8 = mybir.dt.float8e4
AF = mybir.ActivationFunctionType
ALU = mybir.AluOpType
AX = mybir.AxisListType
DR = mybir.MatmulPerfMode.DoubleRow

A_SC = 128.0              # scale on A = Wq^T Wk to dodge fp8 subnormals
QSC = 16.0                # stored-QA scale
WVO_SC = 16.0             # scale on (Wo Wv)^T
WC1_SC = 64.0
WC2_SC = 64.0
WC3_SC = 64.0

# smf (f32 [128, 21]) column map
SMF_BC1 = 0    # 3 cols
SMF_BC2 = 3    # 2 cols
SMF_BF1 = 5    # 6 cols
SMF_VQ = 11    # 6 cols: (bq @ Wk) * QSC
SMF_TS1 = 17   # 1 - 0.5*ts
SMF_TS = 18    # ts
SMF_NBC3 = 19  # -bc3 (bcast)
SMF_NBF2 = 20  # -bf2 (bcast)
SMF_ID = 21    # 128 cols: f32 identity (for f32 PE transposes)
SMF_NEG1 = SMF_ID + 128   # column of -1.0
SMF_N = SMF_NEG1 + 1

# bfp (bf16) column map
BFP_PW = 0                    # 3*WIN pw tables (first/mid/last)
BFP_WF2 = 3 * WIN             # 6 cols
NBF = BFP_WF2 + 6

_cache = {}


def r3(ap, c):
    return ap.rearrange("p (c s) -> p c s", c=c)


def build_kernel(dbg=False):
    nc = bacc.Bacc(None, target_bir_lowering=False)

    def din(name, shape, dt=F32):
        return nc.dram_tensor(name, shape, dt, kind="ExternalInput")

    xpT8d = din("xpT8", [128, HC * NB * 128], F8)  # feature-major (x+pos)
    a8d = din("a8", [H, H], F8)                    # Wq^T Wk * A_SC  [f, g]
    wvo8d = din("wvo8", [H, H], F8)                # (Wo Wv)^T * WVO_SC
    wc18d = din("wc18", [H, 384], F8)              # Wc1^T * WC1_SC
    wc28d = din("wc28", [512, 256], F8)            # Wc2^T * WC2_SC, pad 192->256
    f8pd = din("f8p", [128, 2], F8)                # Wc3 * WC3_SC columns
    wf1d = din("wf1p", [3, H], BF16)               # Wf1^T / S
    smfd = din("smf", [128, SMF_N], F32)
    bfpd = din("bfp", [128, NBF], BF16)            # ident | pw f/m/l | wf2

    y = nc.dram_tensor("y", [1024, H], BF16, kind="ExternalOutput")
    zrd = nc.dram_tensor("zr8", [128, 8], F32, kind="ExternalOutput")

    with tile.TileContext(nc) as tc:
        with (
            tc.tile_pool(name="pers", bufs=1) as pers,
            tc.tile_pool(name="pQ", bufs=2, space="PSUM") as pQ,
            tc.tile_pool(name="pB", bufs=4, space="PSUM") as pB,
            tc.tile_pool(name="pC", bufs=2, space="PSUM") as pC,
        ):
            # ---------- startup PE warmup (no data dependencies) ----------
            scr0 = pers.tile([128, 128], F32, tag="scr0")
            nc.gpsimd.memset(scr0[:], 0)
            for _ in range(12):
                pd = pC.tile([128, 128], F32, tag="pc")
                nc.tensor.transpose(pd[:], scr0[:], scr0[:])

            # ---------- DMA loads (wave 0: what QA needs first) ----------
            a8 = pers.tile([128, HC * H], F8, tag="a8")
            ha = HC * H // 2
            nc.sync.dma_start(
                r3(a8[:, 0:ha], 3),
                a8d[0:H // 2, :].rearrange("(c p) n -> p c n", c=3))
            nc.scalar.dma_start(
                r3(a8[:, ha:2 * ha], 3),
                a8d[H // 2:H, :].rearrange("(c p) n -> p c n", c=3))
            xpT8 = pers.tile([128, HC * NB * 128], F8, tag="xpT8")
            h3 = HC * NB * 128 // 3
            smf_t = pers.tile([128, SMF_N], F32, tag="smf")
            nc.gpsimd.dma_start(smf_t[:], smfd[:])
            nc.sync.dma_start(xpT8[:, 0:h3], xpT8d[:, 0:h3])
            nc.sync.dma_start(xpT8[:, h3:2 * h3], xpT8d[:, h3:2 * h3])
            nc.gpsimd.dma_start(xpT8[:, 2 * h3:3 * h3], xpT8d[:, 2 * h3:3 * h3])
            bfp_t = pers.tile([128, NBF], BF16, tag="bfp")
            nc.gpsimd.dma_start(bfp_t[:], bfpd[:])
            identb32 = smf_t[:, SMF_ID:SMF_ID + 128]
            pw_at = {1: BFP_PW, 8: BFP_PW + 2 * WIN}  # t -> col (else mid)
            wf2_b = bfp_t[:, BFP_WF2:BFP_WF2 + 6]
            # wave 1
            wvo8 = pers.tile([128, HC * H], F8, tag="wvo8")
            nc.sync.dma_start(
                r3(wvo8[:], HC), wvo8d[:].rearrange("(c p) n -> p c n", c=HC))
            wc18 = pers.tile([128, HC * 384], F8, tag="wc18")
            nc.sync.dma_start(
                r3(wc18[:], HC), wc18d[:].rearrange("(c p) n -> p c n", c=HC))
            wc28 = pers.tile([128, 4 * 256], F8, tag="wc28")
            nc.gpsimd.dma_start(
                r3(wc28[:], 4), wc28d[:].rearrange("(c p) n -> p c n", c=4))
            f8p_t = pers.tile([128, 2], F8, tag="f8p")
            nc.gpsimd.dma_start(f8p_t[:], f8pd[:])
            wf1_t = pers.tile([3, H], BF16, tag="wf1")
            nc.gpsimd.dma_start(wf1_t[:], wf1d[:])

            xp8 = r3(xpT8[:], HC)
            a8v = r3(a8[:], HC)
            wvov = r3(wvo8[:], HC)
            wc1v = r3(wc18[:], HC)
            wc2v = r3(wc28[:], 4)

            # ---------- persistent activations ----------
            qa8 = pers.tile([128, HC * 1024], F8, tag="qa8")
            h1T = pers.tile([128, 4 * 1024], F8, tag="h1T")
            h2T = pers.tile([128, 2 * 1024], F8, tag="h2T")
            fu1s = pers.tile([128, HC * 1024], BF16, tag="fu1s")
            vw = pers.tile([128, NB * H], BF16, tag="vw")
            bp8 = pers.tile([128, 8 * WIN], BF16, tag="bp8")
            rs8 = pers.tile([128, 8], F32, tag="rs8")
            ci8 = pers.tile([128, 8], F32, tag="ci8")
            fw8 = pers.tile([128, 8], F32, tag="fw8")
            g8 = pers.tile([128, 8], F32, tag="g8")
            eci = pers.tile([128, 8], F32, tag="eci")
            efw = pers.tile([128, 8], F32, tag="efw")
            pre8 = pers.tile([128, 8], F32, tag="pre8")
            zr8 = pers.tile([128, 8], F32, tag="zr8")
            ftin = pers.tile([128, 24], F32, tag="ftin")
            fin = pers.tile([3, 1024], BF16, tag="fin")
            qav = r3(qa8[:], HC)
            h1v = r3(h1T[:], 4)
            h2v = r3(h2T[:], 2)
            fuv = r3(fu1s[:], HC)
            vwv = r3(vw[:], NB)

            # ---------- phase helpers ----------
            def qa_wave(j2):
                for oc in range(HC):
                    p = pQ.tile([128, 512], F32, tag="pq")
                    for c in range(3):
                        nc.tensor.matmul(
                            p[:],
                            a8v[:, 2 * c:2 * c + 2, oc * 128:(oc + 1) * 128],
                            xp8[:, 2 * c:2 * c + 2,
                                128 + j2 * 512:128 + (j2 + 1) * 512],
                            start=(c == 0), stop=(c == 2),
                            perf_mode=DR, skip_group_check=True)
                    dst = qa8[:, oc * 1024 + j2 * 512:
                              oc * 1024 + (j2 + 1) * 512]
                    vq = smf_t[:, SMF_VQ + oc:SMF_VQ + oc + 1]
                    if oc % 2 == 0:
                        nc.vector.tensor_scalar(dst, p[:], QSC / A_SC, vq,
                                                ALU.mult, ALU.add)
                    else:
                        nc.scalar.activation(dst, p[:], AF.Identity, bias=vq,
                                             scale=QSC / A_SC)

            def score_block(t):
                p = pB.tile([128, WIN], F32, tag="pb")
                for n0, nn in ((0, 256), (256, 128)):
                    for c in range(3):
                        nc.tensor.matmul(
                            p[:, n0:n0 + nn],
                            qav[:, 2 * c:2 * c + 2, (t - 1) * 128:t * 128],
                            xp8[:, 2 * c:2 * c + 2,
                                (t - 1) * 128 + n0:(t - 1) * 128 + n0 + nn],
                            start=(c == 0), stop=(c == 2),
                            perf_mode=DR, skip_group_check=True)
                pwc = pw_at.get(t, BFP_PW + WIN)
                nc.vector.tensor_tensor(bp8[:, (t - 1) * WIN:t * WIN],
                                        p[:], bfp_t[:, pwc:pwc + WIN],
                                        ALU.mult)
                nc.vector.tensor_reduce(rs8[:, t - 1:t],
                                        bp8[:, (t - 1) * WIN:t * WIN],
                                        AX.X, ALU.add)

            def h1_wave(j2):
                for oc in range(3):
                    p = pQ.tile([128, 512], F32, tag="pq")
                    for c in range(3):
                        nc.tensor.matmul(
                            p[:],
                            wc1v[:, 2 * c:2 * c + 2, oc * 128:(oc + 1) * 128],
                            xp8[:, 2 * c:2 * c + 2,
                                128 + j2 * 512:128 + (j2 + 1) * 512],
                            start=(c == 0), stop=(c == 2),
                            perf_mode=DR, skip_group_check=True)
                    nc.scalar.activation(
                        h1T[:, oc * 1024 + j2 * 512:
                            oc * 1024 + (j2 + 1) * 512],
                        p[:], AF.Relu, scale=1.0 / WC1_SC,
                        bias=smf_t[:, SMF_BC1 + oc:SMF_BC1 + oc + 1])

            def h2_wave(j2):
                for oc in range(2):
                    p = pQ.tile([128, 512], F32, tag="pq")
                    for c2 in range(2):
                        nc.tensor.matmul(
                            p[:],
                            wc2v[:, 2 * c2:2 * c2 + 2, oc * 128:(oc + 1) * 128],
                            h1v[:, 2 * c2:2 * c2 + 2,
                                j2 * 512:(j2 + 1) * 512],
                            start=(c2 == 0), stop=(c2 == 1),
                            perf_mode=DR, skip_group_check=True)
                    nc.scalar.activation(
                        h2T[:, oc * 1024 + j2 * 512:
                            oc * 1024 + (j2 + 1) * 512],
                        p[:], AF.Relu, scale=1.0 / WC2_SC,
                        bias=smf_t[:, SMF_BC2 + oc:SMF_BC2 + oc + 1])

            ci_ps = pC.tile([128, 128], F32, tag="pc")

            def ci_cols(ts_):
                for t in ts_:
                    for cc in range(2):
                        nc.tensor.matmul(
                            ci_ps[:, t:t + 1],
                            h2v[:, cc, t * 128:(t + 1) * 128],
                            f8p_t[:, cc:cc + 1],
                            start=(cc == 0), stop=(cc == 1),
                            skip_group_check=True)

            def vw_block(u, dve_only=False):
                for n2 in range(2):
                    p = pB.tile([128, WIN], F32, tag="pb")
                    for n0, nn in ((0, 256), (256, 128)):
                        for c in range(3):
                            nc.tensor.matmul(
                                p[:, n0:n0 + nn],
                                xp8[:, 2 * c:2 * c + 2, u * 128:(u + 1) * 128],
                                wvov[:, 2 * c:2 * c + 2,
                                     n2 * WIN + n0:n2 * WIN + n0 + nn],
                                start=(c == 0), stop=(c == 2),
                                perf_mode=DR, skip_group_check=True)
                    dst = vw[:, u * H + n2 * WIN:u * H + (n2 + 1) * WIN]
                    if dve_only or (u * 2 + n2) % 2 == 0:
                        nc.vector.tensor_scalar_mul(dst, p[:], 1.0 / WVO_SC)
                    else:
                        nc.scalar.activation(dst, p[:], AF.Identity,
                                             scale=1.0 / WVO_SC)

            ftv = ftin[:].rearrange("p (t k) -> p t k", t=8)

            def fusion_in():
                nc.scalar.activation(eci[:], ci_ps[:, 0:8], AF.Exp,
                                     scale=-1.0 / WC3_SC,
                                     bias=smf_t[:, SMF_NBC3:SMF_NBC3 + 1])
                nc.vector.tensor_scalar_add(eci[:], eci[:], 1.0)
                nc.vector.reciprocal(ci8[:], eci[:])
                nc.vector.tensor_copy(ftv[:, :, 0], rs8[:])
                nc.vector.tensor_scalar_mul(ftv[:, :, 1], rs8[:],
                                            smf_t[:, SMF_TS:SMF_TS + 1])
                nc.vector.tensor_tensor(ftv[:, :, 2], rs8[:], ci8[:],
                                        ALU.mult)
                nc.vector.tensor_scalar(pre8[:], ci8[:], -0.5,
                                        smf_t[:, SMF_TS1:SMF_TS1 + 1],
                                        ALU.mult, ALU.add)
                for t in range(8):
                    pf = pC.tile([128, 128], F32, tag="pc")
                    nc.tensor.transpose(pf[0:3, :], ftin[:, 3 * t:3 * t + 3],
                                        identb32)
                    if t % 2 == 0:
                        nc.vector.tensor_copy(fin[:, t * 128:(t + 1) * 128],
                                              pf[0:3, :])
                    else:
                        nc.scalar.activation(fin[:, t * 128:(t + 1) * 128],
                                             pf[0:3, :], AF.Identity)

            def fu1_oc(oc):
                for hh in range(2):
                    p = pQ.tile([128, 512], F32, tag="pq")
                    nc.tensor.matmul(
                        p[:], wf1_t[0:3, oc * 128:(oc + 1) * 128],
                        fin[:, hh * 512:(hh + 1) * 512],
                        start=True, stop=True, skip_group_check=True)
                    dst = fu1s[:, oc * 1024 + hh * 512:
                               oc * 1024 + (hh + 1) * 512]
                    bcol = smf_t[:, SMF_BF1 + oc:SMF_BF1 + oc + 1]
                    if (oc + hh) % 2 == 0:
                        nc.vector.tensor_scalar(dst, p[:], bcol, 0.0,
                                                ALU.add, ALU.max)
                    else:
                        nc.scalar.activation(dst, p[:], AF.Relu,
                                             bias=bcol)

            def fusion_out():
                fw_ps = pC.tile([128, 128], F32, tag="pc")
                for t in range(8):
                    for oc in range(HC):
                        nc.tensor.matmul(
                            fw_ps[:, t:t + 1],
                            fuv[:, oc, t * 128:(t + 1) * 128],
                            wf2_b[:, oc:oc + 1],
                            start=(oc == 0), stop=(oc == HC - 1),
                            skip_group_check=True)
                nc.scalar.activation(efw[:], fw_ps[:, 0:8], AF.Exp,
                                     scale=-1.0,
                                     bias=smf_t[:, SMF_NBF2:SMF_NBF2 + 1])
                nc.vector.tensor_scalar_add(efw[:], efw[:], 1.0)
                nc.vector.reciprocal(fw8[:], efw[:])
                nc.vector.tensor_tensor(g8[:], fw8[:], pre8[:], ALU.mult)
                nc.vector.tensor_scalar(g8[:], g8[:], -1.0, 1.0,
                                        ALU.mult, ALU.add)

            def pe_warm(n):
                # dependency-free matmuls that keep the PE p-state ramped
                # while the first tail head flows through Act/DVE/SP
                for _ in range(n):
                    pd = pC.tile([128, 128], F32, tag="pc")
                    nc.tensor.transpose(pd[:], smf_t[:, SMF_ID:SMF_ID + 128],
                                        identb32)

            # ---------- tail ops ----------
            with (
                tc.tile_pool(name="eb", bufs=8) as eb,
                tc.tile_pool(name="epb", bufs=8) as epb,
                tc.tile_pool(name="etp", bufs=8) as etp,
                tc.tile_pool(name="zrp", bufs=24) as zrp,
                tc.tile_pool(name="ytp", bufs=8) as ytp,
            ):
                heads = {}

                def tail_head(t):
                    e_t = eb.tile([128, WIN], BF16, tag="et")
                    zc = zrp.tile([128, 1], F32, tag="zc")
                    nc.scalar.activation(e_t[:], bp8[:, (t - 1) * WIN:t * WIN],
                                         AF.Exp, scale=g8[:, t - 1:t],
                                         accum_out=zc[:])
                    ep = epb.tile([128, WIN], BF16, tag="ep")
                    nc.vector.tensor_scalar_add(ep[:], e_t[:], -1.0)
                    zd = zrp.tile([128, 1], F32, tag="zd")
                    nc.vector.tensor_scalar_add(zd[:], zc[:], float(S - WIN))
                    nc.vector.reciprocal(zr8[:, t - 1:t], zd[:])
                    eT = etp.tile([128, 3 * 128], BF16, tag="eT")
                    eTv = r3(eT[:], 3)
                    nc.sync.dma_start(eTv[:], ep[:], transpose=True)
                    heads[t] = eTv

                def tail_y(t, n2_act=True):
                    eTv = heads.pop(t)
                    zrc = zr8[:, t - 1:t]
                    y_t = ytp.tile([128, H], BF16, tag="yt")
                    for n2 in range(2):
                        p = pB.tile([128, WIN], F32, tag="pb")
                        for w in range(3):
                            nc.tensor.matmul(
                                p[:], eTv[:, w, :],
                                vwv[:, t - 1 + w, n2 * WIN:(n2 + 1) * WIN],
                                start=(w == 0), stop=(w == 2),
                                skip_group_check=True)
                        dst = y_t[:, n2 * WIN:(n2 + 1) * WIN]
                        if n2 == 1 and n2_act:
                            nc.scalar.activation(dst, p[:], AF.Identity,
                                                 scale=zrc)
                        else:
                            nc.vector.tensor_scalar_mul(dst, p[:], zrc)
                    nc.gpsimd.dma_start(y[(t - 1) * 128:t * 128, :], y_t[:])

                # ---------- emission schedule ----------
                qa_wave(0)
                qa_wave(1)
                for t in range(1, 9):
                    score_block(t)
                h1_wave(0)
                h1_wave(1)
                h2_wave(0)
                h2_wave(1)
                ci_cols(range(0, 8))
                for u in range(0, NB):
                    vw_block(u)
                fusion_in()
                for oc in range(HC):
                    fu1_oc(oc)
                fusion_out()
                tail_head(1)
                tail_head(2)
                tail_head(3)
                tail_head(4)
                pe_warm(36)
                tail_y(1)
                tail_head(5)
                tail_y(2)
                tail_head(6)
                tail_y(3)
                tail_head(7)
                tail_y(4)
                tail_head(8)
                nc.sync.dma_start(zrd[:], zr8[:])
                for t in range(5, 9):
                    tail_y(t)

    nc.compile()
    return nc


def prep_inputs(x, task_id, pos_emb, Wq, bq, Wk, bk, Wv, bv, Wo, bo,
                task_table, Wt1, bt1, Wt2, bt2,
                Wc1, bc1, Wc2, bc2, Wc3, bc3,
                Wf1, bf1, Wf2, bf2):
    f = np.float32
    bf = ml_dtypes.bfloat16
    f8 = mybir.dt.np(F8)
    T = lambda a: np.ascontiguousarray(np.asarray(a, dtype=f).T)
    c8 = lambda a: np.clip(np.asarray(a, dtype=f), -240, 240).astype(f8)
    sig = lambda a: 1.0 / (1.0 + np.exp(-a))

    Wq_, Wk_, Wv_, Wo_ = (np.asarray(w, f) for w in (Wq, Wk, Wv, Wo))
    wvo = T(Wv_) @ T(Wo_)                    # (Wo Wv)^T
    A = Wq_.T @ Wk_                          # [f, g]
    vq = (np.asarray(bq, f) @ Wk_) * QSC     # [768]

    smf = np.zeros((128, SMF_N), f)
    smf[:, SMF_ID:SMF_ID + 128] = np.eye(128, dtype=f)
    smf[:, SMF_NEG1] = -1.0
    smf[:, SMF_BC1:SMF_BC1 + 3] = np.asarray(bc1, f).reshape(3, 128).T
    bc2p = np.zeros(256, f); bc2p[:192] = np.asarray(bc2, f)
    smf[:, SMF_BC2:SMF_BC2 + 2] = bc2p.reshape(2, 128).T
    smf[:, SMF_BF1:SMF_BF1 + 6] = np.asarray(bf1, f).reshape(HC, 128).T
    smf[:, SMF_VQ:SMF_VQ + 6] = vq.reshape(HC, 128).T
    smf[:, SMF_NBC3] = -float(np.asarray(bc3, f).reshape(-1)[0])
    smf[:, SMF_NBF2] = -float(np.asarray(bf2, f).reshape(-1)[0])

    # pw tables: pos_w / (sqrt(H) * QSC); first/last mask sequence edges
    p_ = np.arange(128)[:, None]; c_ = np.arange(WIN)[None, :]
    pwm = (np.exp(-np.abs(128 + p_ - c_) / 2.0) / (math.sqrt(H) * QSC)).astype(f)

    wc2p = np.zeros((512, 256), f); wc2p[:384, :192] = T(Wc2)
    wc3p = np.zeros((256,), f); wc3p[:192] = np.asarray(Wc3, f).reshape(-1)
    f8pk = (wc3p.reshape(2, 128).T) * WC3_SC

    wf1p = (T(Wf1) / S).astype(bf)                       # [3, 768]
    bvoo = (np.asarray(bv, f) @ Wo_.T + np.asarray(bo, f)).reshape(H)

    # per-batch task scalar
    task_table = np.asarray(task_table, f); task_id = np.asarray(task_id)
    te = task_table[task_id.astype(np.int64)]            # [B, 64]
    t1 = np.maximum(te @ np.asarray(Wt1, f).T + np.asarray(bt1, f), 0.0)
    tw = sig(t1 @ np.asarray(Wt2, f).T + np.asarray(bt2, f))
    ts_all = tw.mean(-1)                                 # [B]

    shared = {
        "a8": c8(A * A_SC), "wvo8": c8(wvo * WVO_SC),
        "wc18": c8(T(Wc1) * WC1_SC), "wc28": c8(wc2p * WC2_SC),
        "f8p": c8(f8pk), "wf1p": wf1p,
    }

    x = np.asarray(x, f); pos_emb = np.asarray(pos_emb, f)
    in_maps = []
    vswo_all = []
    for core in range(8):
        b, half = core // 2, core % 2
        xp_full = x[b] + pos_emb[0]                      # [S, H]
        g0 = 1024 * half - 128
        xp = np.zeros((NB * 128, H), f)
        lo, hi = max(0, g0), min(S, g0 + NB * 128)
        xp[lo - g0:hi - g0] = xp_full[lo:hi]
        # feature-major fp8: [128, HC, NB*128]
        xpT = xp.T.reshape(HC, 128, NB * 128).transpose(1, 0, 2)
        xpT8 = c8(xpT.reshape(128, HC * NB * 128))

        pwf = pwm.copy(); pwl = pwm.copy()
        if half == 0:
            pwf[:, :128] = 0
        if half == 1:
            pwl[:, 256:] = 0
        bfpk = np.zeros((128, NBF), bf)
        bfpk[:, BFP_PW:BFP_PW + WIN] = pwf.astype(bf)
        bfpk[:, BFP_PW + WIN:BFP_PW + 2 * WIN] = pwm.astype(bf)
        bfpk[:, BFP_PW + 2 * WIN:BFP_PW + 3 * WIN] = pwl.astype(bf)
        bfpk[:, BFP_WF2:BFP_WF2 + 6] = \
            np.asarray(Wf2, f).reshape(HC, 128).T.astype(bf)

        smfc = smf.copy()
        ts = float(ts_all[b])
        smfc[:, SMF_TS1] = 1.0 - 0.5 * ts
        smfc[:, SMF_TS] = ts

        m = dict(shared)
        m.update({"xpT8": xpT8, "bfp": bfpk, "smf": smfc})
        in_maps.append(m)
        vswo_all.append((xp_full.sum(0) @ wvo).astype(f))
    return in_maps, bvoo, vswo_all


class _Runner:
    """Compile the SPMD graph once and keep a reusable jitted callable."""

    def __init__(self, nc, n_cores=8):
        import jax
        from jax.sharding import Mesh, PartitionSpec
        from jax.experimental.shard_map import shard_map
        from concourse import bass2jax, mybir as _mb
        bass2jax.install_neuronx_cc_hook()
        self.nc = nc
        partition_name = (nc.partition_id_tensor.name
                          if nc.partition_id_tensor else None)
        in_names, out_names, out_avals, zero_outs = [], [], [], []
        for alloc in nc.m.functions[0].allocations:
            if not isinstance(alloc, _mb.MemoryLocationSet):
                continue
            name = alloc.memorylocations[0].name
            if alloc.kind == "ExternalInput":
                if name != partition_name:
                    in_names.append(name)
            elif alloc.kind == "ExternalOutput":
                shape = tuple(alloc.tensor_shape)
                dtype = _mb.dt.np(alloc.dtype)
                out_names.append(name)
                out_avals.append(jax.core.ShapedArray(shape, dtype))
                zero_outs.append(np.zeros(shape, dtype))
        self.in_names = list(in_names)
        self.out_names = out_names
        self.out_avals = out_avals
        self.zero_outs = zero_outs
        self.n_cores = n_cores
        n_params = len(self.in_names)
        all_in = list(self.in_names) + list(out_names)
        if partition_name is not None:
            all_in.append(partition_name)

        def _body(*args):
            operands = list(args)
            if partition_name is not None:
                operands.append(bass2jax.partition_id_tensor())
            outs = bass2jax._bass_exec_p.bind(
                *operands,
                out_avals=tuple(out_avals),
                in_names=tuple(all_in),
                out_names=tuple(out_names),
                lowering_input_output_aliases=(),
                sim_require_finite=True,
                sim_require_nnan=True,
                nc=nc,
            )
            return tuple(outs)

        devices = jax.devices()[:n_cores]
        mesh = Mesh(np.asarray(devices), ("core",))
        n_outs = len(out_names)
        in_specs = (PartitionSpec("core"),) * (n_params + n_outs)
        out_specs = (PartitionSpec("core"),) * n_outs
        self.fn = jax.jit(
            shard_map(_body, mesh=mesh, in_specs=in_specs,
                      out_specs=out_specs, check_rep=False),
            keep_unused=True)

    def concat_inputs(self, in_maps):
        return [np.concatenate([np.asarray(in_maps[c][k])
                                for c in range(self.n_cores)], axis=0)
                for k in self.in_names]

    def zeros(self):
        return [np.zeros((self.n_cores * z.shape[0],) + z.shape[1:], z.dtype)
                for z in self.zero_outs]

    def __call__(self, concat_in, zeros=None):
        import jax
        if zeros is None:
            zeros = self.zeros()
        outs = jax.block_until_ready(self.fn(*concat_in, *zeros))
        return outs


def get_runner(dbg=False):
    if "runner" not in _cache:
        _cache["runner"] = _Runner(build_kernel())
    return _cache["runner"]


def kernel(**inputs):
    rn = get_runner()
    in_maps, bvoo, vswo_all = prep_inputs(**inputs)
    outs = rn(rn.concat_inputs(in_maps))
    yc = np.asarray(outs[rn.out_names.index("y")]).reshape(8, 1024, H)
    zc = np.asarray(outs[rn.out_names.index("zr8")]).reshape(8, 128, 8)
    out = np.zeros((B, S, H), np.float32)
    for core in range(8):
        b, half = core // 2, core % 2
        zr = zc[core].T.reshape(1024, 1)
        out[b, 1024 * half:1024 * (half + 1)] = \
            yc[core].astype(np.float32) + zr * vswo_all[core] + bvoo
    return out


# revision 76
# speedup vs baseline: 1.2142x; 1.0084x over previous
"""Trainium2 Bass kernel for AdaptivePositionAwareAttention.

Banded softmax + closed-form far field: pos_w = exp(-|i-j|/2) kills
exp(f)-1 beyond ~90 positions, so per 128-row query block t only key
blocks t-1..t+1 (WIN=384) matter:

    y_i = zr_i * (VsumWo + sum_band (e^{f_ij}-1) VW_j) + (bv@Wo + bo)

where VW = xp @ (Wo Wv)^T, VsumWo = colsum of VW over the full sequence,
and zr_i = 1/(S - WIN + sum_band e^f).  The task/content/fusion branches
reduce to a per-row scalar g with f = base * g.

v2 layout: the host pre-adds pos, pre-transposes x to feature-major fp8
(xpT8), pre-computes the far-field row (vswo = colsum(xp) @ WvoT), the
per-batch task scalar, and A = Wq^T Wk so no K projection is needed:
scores = (xp@A) @ xp^T, both matmuls fp8 DoubleRow.  ci / fw use
free-stationary column matmuls ([128,1] outputs); sigmoids run as
exp + reciprocal so the Activation engine never reloads its table;
E'^T is produced by DMA-XBAR transposes on the SP queue.

Sharding: 8 cores = (batch 0..3) x (sequence half); each core owns 1024
query rows plus one 128-row zero-padded halo block per side.  Sequence
edges ride on per-core masked pos_w data, so all cores run one SPMD
graph.
"""

import math

import numpy as np
import ml_dtypes

import concourse.bass as bass
from concourse import bacc, mybir, tile

# tensor_tensor_reduce and mixed DR/non-DR psum groups crash real HW
# (NRT_EXEC_UNIT_UNRECOVERABLE) though CoreSim accepts them — keep off.
USE_TTR = False
USE_H2DR = False

B, S, H = 4, 2048, 768
HC = H // 128             # 6 feature chunks
NB = 10                   # blocks per core incl. halo (1280 rows)
WIN = 384                 # key window = 3 blocks
F32 = mybir.dt.float32
BF16 = mybir.dt.bfloat16
F8 = mybir.dt.float8e4
AF = mybir.ActivationFunctionType
ALU = mybir.AluOpType
AX = mybir.AxisListType
DR = mybir.MatmulPerfMode.DoubleRow

A_SC = 128.0              # scale on A = Wq^T Wk to dodge fp8 subnormals
QSC = 16.0                # stored-QA scale
WVO_SC = 16.0             # scale on (Wo Wv)^T
WC1_SC = 64.0
WC2_SC = 64.0
WC3_SC = 64.0

# smf (f32 [128, 21]) column map
SMF_BC1 = 0    # 3 cols
SMF_BC2 = 3    # 2 cols
SMF_BF1 = 5    # 6 cols
SMF_VQ = 11    # 6 cols: (bq @ Wk) * QSC
SMF_TS1 = 17   # 1 - 0.5*ts
SMF_TS = 18    # ts
SMF_NBC3 = 19  # -bc3 (bcast)
SMF_NBF2 = 20  # -bf2 (bcast)
SMF_ID = 21    # 128 cols: f32 identity (for f32 PE transposes)
SMF_NEG1 = SMF_ID + 128   # column of -1.0
SMF_N = SMF_NEG1 + 1

# bfp (bf16) column map
BFP_PW = 0                    # 3*WIN pw tables (first/mid/last)
BFP_WF2 = 3 * WIN             # 6 cols
NBF = BFP_WF2 + 6

_cache = {}


def r3(ap, c):
    return ap.rearrange("p (c s) -> p c s", c=c)


def build_kernel(dbg=False):
    nc = bacc.Bacc(None, target_bir_lowering=False)

    def din(name, shape, dt=F32):
        return nc.dram_tensor(name, shape, dt, kind="ExternalInput")

    xpT8d = din("xpT8", [128, HC * NB * 128], F8)  # feature-major (x+pos)
    a8d = din("a8", [H, H], F8)                    # Wq^T Wk * A_SC  [f, g]
    wvo8d = din("wvo8", [H, H], F8)                # (Wo Wv)^T * WVO_SC
    wc18d = din("wc18", [H, 384], F8)              # Wc1^T * WC1_SC
    wc28d = din("wc28", [512, 256], F8)            # Wc2^T * WC2_SC, pad 192->256
    f8pd = din("f8p", [128, 2], F8)                # Wc3 * WC3_SC columns
    wf1d = din("wf1p", [3, H], BF16)               # Wf1^T / S
    smfd = din("smf", [128, SMF_N], F32)
    bfpd = din("bfp", [128, NBF], BF16)            # ident | pw f/m/l | wf2

    y = nc.dram_tensor("y", [1024, H], BF16, kind="ExternalOutput")
    zrd = nc.dram_tensor("zr8", [128, 8], F32, kind="ExternalOutput")

    with tile.TileContext(nc) as tc:
        with (
            tc.tile_pool(name="pers", bufs=1) as pers,
            tc.tile_pool(name="pQ", bufs=2, space="PSUM") as pQ,
            tc.tile_pool(name="pB", bufs=4, space="PSUM") as pB,
            tc.tile_pool(name="pC", bufs=2, space="PSUM") as pC,
        ):
            # ---------- startup PE warmup (no data dependencies) ----------
            scr0 = pers.tile([128, 128], F32, tag="scr0")
            nc.gpsimd.memset(scr0[:], 0)
            for _ in range(12):
                pd = pC.tile([128, 128], F32, tag="pc")
                nc.tensor.transpose(pd[:], scr0[:], scr0[:])

            # ---------- DMA loads (wave 0: what QA needs first) ----------
            a8 = pers.tile([128, HC * H], F8, tag="a8")
            ha = HC * H // 2
            nc.sync.dma_start(
                r3(a8[:, 0:ha], 3),
                a8d[0:H // 2, :].rearrange("(c p) n -> p c n", c=3))
            nc.scalar.dma_start(
                r3(a8[:, ha:2 * ha], 3),
                a8d[H // 2:H, :].rearrange("(c p) n -> p c n", c=3))
            xpT8 = pers.tile([128, HC * NB * 128], F8, tag="xpT8")
            h3 = HC * NB * 128 // 3
            smf_t = pers.tile([128, SMF_N], F32, tag="smf")
            nc.gpsimd.dma_start(smf_t[:], smfd[:])
            nc.sync.dma_start(xpT8[:, 0:h3], xpT8d[:, 0:h3])
            nc.sync.dma_start(xpT8[:, h3:2 * h3], xpT8d[:, h3:2 * h3])
            nc.gpsimd.dma_start(xpT8[:, 2 * h3:3 * h3], xpT8d[:, 2 * h3:3 * h3])
            bfp_t = pers.tile([128, NBF], BF16, tag="bfp")
            nc.gpsimd.dma_start(bfp_t[:], bfpd[:])
            identb32 = smf_t[:, SMF_ID:SMF_ID + 128]
            pw_at = {1: BFP_PW, 8: BFP_PW + 2 * WIN}  # t -> col (else mid)
            wf2_b = bfp_t[:, BFP_WF2:BFP_WF2 + 6]
            # wave 1
            wvo8 = pers.tile([128, HC * H], F8, tag="wvo8")
            nc.sync.dma_start(
                r3(wvo8[:], HC), wvo8d[:].rearrange("(c p) n -> p c n", c=HC))
            wc18 = pers.tile([128, HC * 384], F8, tag="wc18")
            nc.sync.dma_start(
                r3(wc18[:], HC), wc18d[:].rearrange("(c p) n -> p c n", c=HC))
            wc28 = pers.tile([128, 4 * 256], F8, tag="wc28")
            nc.gpsimd.dma_start(
                r3(wc28[:], 4), wc28d[:].rearrange("(c p) n -> p c n", c=4))
            f8p_t = pers.tile([128, 2], F8, tag="f8p")
            nc.gpsimd.dma_start(f8p_t[:], f8pd[:])
            wf1_t = pers.tile([3, H], BF16, tag="wf1")
            nc.gpsimd.dma_start(wf1_t[:], wf1d[:])

            xp8 = r3(xpT8[:], HC)
            a8v = r3(a8[:], HC)
            wvov = r3(wvo8[:], HC)
            wc1v = r3(wc18[:], HC)
            wc2v = r3(wc28[:], 4)

            # ---------- persistent activations ----------
            qa8 = pers.tile([128, HC * 1024], F8, tag="qa8")
            h1T = pers.tile([128, 4 * 1024], F8, tag="h1T")
            h2T = pers.tile([128, 2 * 1024], F8, tag="h2T")
            fu1s = pers.tile([128, HC * 1024], BF16, tag="fu1s")
            vw = pers.tile([128, NB * H], BF16, tag="vw")
            bp8 = pers.tile([128, 8 * WIN], BF16, tag="bp8")
            rs8 = pers.tile([128, 8], F32, tag="rs8")
            ci8 = pers.tile([128, 8], F32, tag="ci8")
            fw8 = pers.tile([128, 8], F32, tag="fw8")
            g8 = pers.tile([128, 8], F32, tag="g8")
            eci = pers.tile([128, 8], F32, tag="eci")
            efw = pers.tile([128, 8], F32, tag="efw")
            pre8 = pers.tile([128, 8], F32, tag="pre8")
            zr8 = pers.tile([128, 8], F32, tag="zr8")
            ftin = pers.tile([128, 24], F32, tag="ftin")
            fin = pers.tile([3, 1024], BF16, tag="fin")
            qav = r3(qa8[:], HC)
            h1v = r3(h1T[:], 4)
            h2v = r3(h2T[:], 2)
            fuv = r3(fu1s[:], HC)
            vwv = r3(vw[:], NB)

            # ---------- phase helpers ----------
            def qa_wave(j2):
                for oc in range(HC):
                    p = pQ.tile([128, 512], F32, tag="pq")
                    for c in range(3):
                        nc.tensor.matmul(
                            p[:],
                            a8v[:, 2 * c:2 * c + 2, oc * 128:(oc + 1) * 128],
                            xp8[:, 2 * c:2 * c + 2,
                                128 + j2 * 512:128 + (j2 + 1) * 512],
                            start=(c == 0), stop=(c == 2),
                            perf_mode=DR, skip_group_check=True)
                    dst = qa8[:, oc * 1024 + j2 * 512:
                              oc * 1024 + (j2 + 1) * 512]
                    vq = smf_t[:, SMF_VQ + oc:SMF_VQ + oc + 1]
                    if oc % 2 == 0:
                        nc.vector.tensor_scalar(dst, p[:], QSC / A_SC, vq,
                                                ALU.mult, ALU.add)
                    else:
                        nc.scalar.activation(dst, p[:], AF.Identity, bias=vq,
                                             scale=QSC / A_SC)

            def score_block(t):
                p = pB.tile([128, WIN], F32, tag="pb")
                for n0, nn in ((0, 256), (256, 128)):
                    for c in range(3):
                        nc.tensor.matmul(
                            p[:, n0:n0 + nn],
                            qav[:, 2 * c:2 * c + 2, (t - 1) * 128:t * 128],
                            xp8[:, 2 * c:2 * c + 2,
                                (t - 1) * 128 + n0:(t - 1) * 128 + n0 + nn],
                            start=(c == 0), stop=(c == 2),
                            perf_mode=DR, skip_group_check=True)
                pwc = pw_at.get(t, BFP_PW + WIN)
                nc.vector.tensor_tensor(bp8[:, (t - 1) * WIN:t * WIN],
                                        p[:], bfp_t[:, pwc:pwc + WIN],
                                        ALU.mult)
                nc.vector.tensor_reduce(rs8[:, t - 1:t],
                                        bp8[:, (t - 1) * WIN:t * WIN],
                                        AX.X, ALU.add)

            def h1_wave(j2):
                for oc in range(3):
                    p = pQ.tile([128, 512], F32, tag="pq")
                    for c in range(3):
                        nc.tensor.matmul(
                            p[:],
                            wc1v[:, 2 * c:2 * c + 2, oc * 128:(oc + 1) * 128],
                            xp8[:, 2 * c:2 * c + 2,
                                128 + j2 * 512:128 + (j2 + 1) * 512],
                            start=(c == 0), stop=(c == 2),
                            perf_mode=DR, skip_group_check=True)
                    nc.scalar.activation(
                        h1T[:, oc * 1024 + j2 * 512:
                            oc * 1024 + (j2 + 1) * 512],
                        p[:], AF.Relu, scale=1.0 / WC1_SC,
                        bias=smf_t[:, SMF_BC1 + oc:SMF_BC1 + oc + 1])

            def h2_wave(j2):
                for oc in range(2):
                    p = pQ.tile([128, 512], F32, tag="pq")
                    for c2 in range(2):
                        nc.tensor.matmul(
                            p[:],
                            wc2v[:, 2 * c2:2 * c2 + 2, oc * 128:(oc + 1) * 128],
                            h1v[:, 2 * c2:2 * c2 + 2,
                                j2 * 512:(j2 + 1) * 512],
                            start=(c2 == 0), stop=(c2 == 1),
                            perf_mode=DR, skip_group_check=True)
                    nc.scalar.activation(
                        h2T[:, oc * 1024 + j2 * 512:
                            oc * 1024 + (j2 + 1) * 512],
                        p[:], AF.Relu, scale=1.0 / WC2_SC,
                        bias=smf_t[:, SMF_BC2 + oc:SMF_BC2 + oc + 1])

            ci_ps = pC.tile([128, 128], F32, tag="pc")

            def ci_cols(ts_):
                for t in ts_:
                    for cc in range(2):
                        nc.tensor.matmul(
                            ci_ps[:, t:t + 1],
                            h2v[:, cc, t * 128:(t + 1) * 128],
                            f8p_t[:, cc:cc + 1],
                            start=(cc == 0), stop=(cc == 1),
                            skip_group_check=True)

            def vw_block(u, dve_only=False):
                for n2 in range(2):
                    p = pB.tile([128, WIN], F32, tag="pb")
                    for n0, nn in ((0, 256), (256, 128)):
                        for c in range(3):
                            nc.tensor.matmul(
                                p[:, n0:n0 + nn],
                                xp8[:, 2 * c:2 * c + 2, u * 128:(u + 1) * 128],
                                wvov[:, 2 * c:2 * c + 2,
                                     n2 * WIN + n0:n2 * WIN + n0 + nn],
                                start=(c == 0), stop=(c == 2),
                                perf_mode=DR, skip_group_check=True)
                    dst = vw[:, u * H + n2 * WIN:u * H + (n2 + 1) * WIN]
                    if dve_only or (u * 2 + n2) % 2 == 0:
                        nc.vector.tensor_scalar_mul(dst, p[:], 1.0 / WVO_SC)
                    else:
                        nc.scalar.activation(dst, p[:], AF.Identity,
                                             scale=1.0 / WVO_SC)

            ftv = ftin[:].rearrange("p (t k) -> p t k", t=8)

            def fusion_in():
                nc.scalar.activation(eci[:], ci_ps[:, 0:8], AF.Exp,
                                     scale=-1.0 / WC3_SC,
                                     bias=smf_t[:, SMF_NBC3:SMF_NBC3 + 1])
                nc.vector.tensor_scalar_add(eci[:], eci[:], 1.0)
                nc.vector.reciprocal(ci8[:], eci[:])
                nc.vector.tensor_copy(ftv[:, :, 0], rs8[:])
                nc.vector.tensor_scalar_mul(ftv[:, :, 1], rs8[:],
                                            smf_t[:, SMF_TS:SMF_TS + 1])
                nc.vector.tensor_tensor(ftv[:, :, 2], rs8[:], ci8[:],
                                        ALU.mult)
                nc.vector.tensor_scalar(pre8[:], ci8[:], -0.5,
                                        smf_t[:, SMF_TS1:SMF_TS1 + 1],
                                        ALU.mult, ALU.add)
                for t in range(8):
                    pf = pC.tile([128, 128], F32, tag="pc")
                    nc.tensor.transpose(pf[0:3, :], ftin[:, 3 * t:3 * t + 3],
                                        identb32)
                    if t % 2 == 0:
                        nc.vector.tensor_copy(fin[:, t * 128:(t + 1) * 128],
                                              pf[0:3, :])
                    else:
                        nc.scalar.activation(fin[:, t * 128:(t + 1) * 128],
                                             pf[0:3, :], AF.Identity)

            def fu1_oc(oc):
                for hh in range(2):
                    p = pQ.tile([128, 512], F32, tag="pq")
                    nc.tensor.matmul(
                        p[:], wf1_t[0:3, oc * 128:(oc + 1) * 128],
                        fin[:, hh * 512:(hh + 1) * 512],
                        start=True, stop=True, skip_group_check=True)
                    dst = fu1s[:, oc * 1024 + hh * 512:
                               oc * 1024 + (hh + 1) * 512]
                    bcol = smf_t[:, SMF_BF1 + oc:SMF_BF1 + oc + 1]
                    if (oc + hh) % 2 == 0:
                        nc.vector.tensor_scalar(dst, p[:], bcol, 0.0,
                                                ALU.add, ALU.max)
                    else:
                        nc.scalar.activation(dst, p[:], AF.Relu,
                                             bias=bcol)

            def fusion_out():
                fw_ps = pC.tile([128, 128], F32, tag="pc")
                for t in range(8):
                    for oc in range(HC):
                        nc.tensor.matmul(
                            fw_ps[:, t:t + 1],
                            fuv[:, oc, t * 128:(t + 1) * 128],
                            wf2_b[:, oc:oc + 1],
                            start=(oc == 0), stop=(oc == HC - 1),
                            skip_group_check=True)
                nc.scalar.activation(efw[:], fw_ps[:, 0:8], AF.Exp,
                                     scale=-1.0,
                                     bias=smf_t[:, SMF_NBF2:SMF_NBF2 + 1])
                nc.vector.tensor_scalar_add(efw[:], efw[:], 1.0)
                nc.vector.reciprocal(fw8[:], efw[:])
                nc.vector.tensor_tensor(g8[:], fw8[:], pre8[:], ALU.mult)
                nc.vector.tensor_scalar(g8[:], g8[:], -1.0, 1.0,
                                        ALU.mult, ALU.add)

            def pe_warm(n):
                # dependency-free matmuls that keep the PE p-state ramped
                # while the first tail head flows through Act/DVE/SP
                for _ in range(n):
                    pd = pC.tile([128, 128], F32, tag="pc")
                    nc.tensor.transpose(pd[:], smf_t[:, SMF_ID:SMF_ID + 128],
                                        identb32)

            # ---------- tail ops ----------
            with (
                tc.tile_pool(name="eb", bufs=8) as eb,
                tc.tile_pool(name="epb", bufs=8) as epb,
                tc.tile_pool(name="etp", bufs=8) as etp,
                tc.tile_pool(name="zrp", bufs=24) as zrp,
                tc.tile_pool(name="ytp", bufs=8) as ytp,
            ):
                heads = {}

                def tail_head(t):
                    e_t = eb.tile([128, WIN], BF16, tag="et")
                    zc = zrp.tile([128, 1], F32, tag="zc")
                    nc.scalar.activation(e_t[:], bp8[:, (t - 1) * WIN:t * WIN],
                                         AF.Exp, scale=g8[:, t - 1:t],
                                         accum_out=zc[:])
                    ep = epb.tile([128, WIN], BF16, tag="ep")
                    nc.vector.tensor_scalar_add(ep[:], e_t[:], -1.0)
                    zd = zrp.tile([128, 1], F32, tag="zd")
                    nc.vector.tensor_scalar_add(zd[:], zc[:], float(S - WIN))
                    nc.vector.reciprocal(zr8[:, t - 1:t], zd[:])
                    eT = etp.tile([128, 3 * 128], BF16, tag="eT")
                    eTv = r3(eT[:], 3)
                    nc.sync.dma_start(eTv[:], ep[:], transpose=True)
                    heads[t] = eTv

                def tail_y(t, n2_act=True):
                    eTv = heads.pop(t)
                    zrc = zr8[:, t - 1:t]
                    y_t = ytp.tile([128, H], BF16, tag="yt")
                    for n2 in range(2):
                        p = pB.tile([128, WIN], F32, tag="pb")
                        for w in range(3):
                            nc.tensor.matmul(
                                p[:], eTv[:, w, :],
                                vwv[:, t - 1 + w, n2 * WIN:(n2 + 1) * WIN],
                                start=(w == 0), stop=(w == 2),
                                skip_group_check=True)
                        dst = y_t[:, n2 * WIN:(n2 + 1) * WIN]
                        if n2 == 1 and n2_act:
                            nc.scalar.activation(dst, p[:], AF.Identity,
                                                 scale=zrc)
                        else:
                            nc.vector.tensor_scalar_mul(dst, p[:], zrc)
                    nc.gpsimd.dma_start(y[(t - 1) * 128:t * 128, :], y_t[:])

                # ---------- emission schedule ----------
                qa_wave(0)
                qa_wave(1)
                for t in range(1, 9):
                    score_block(t)
                h1_wave(0)
                h1_wave(1)
                h2_wave(0)
                h2_wave(1)
                ci_cols(range(0, 8))
                fusion_in()
                for oc in range(HC):
                    fu1_oc(oc)
                fusion_out()
                tail_head(1)
                vw_block(0)
                vw_block(1)
                tail_head(2)
                vw_block(2)
                vw_block(3)
                tail_head(3)
                vw_block(4)
                vw_block(5)
                pe_warm(36)
                tail_y(1)
                tail_head(4)
                vw_block(6)
                vw_block(7)
                tail_y(2)
                tail_head(5)
                vw_block(8)
                vw_block(9)
                tail_y(3)
                for t in range(4, 9):
                    if t + 2 <= 8:
                        tail_head(t + 2)
                        if t + 2 == 8:
                            nc.sync.dma_start(zrd[:], zr8[:])
                    tail_y(t)

    nc.compile()
    return nc


def prep_inputs(x, task_id, pos_emb, Wq, bq, Wk, bk, Wv, bv, Wo, bo,
                task_table, Wt1, bt1, Wt2, bt2,
                Wc1, bc1, Wc2, bc2, Wc3, bc3,
                Wf1, bf1, Wf2, bf2):
    f = np.float32
    bf = ml_dtypes.bfloat16
    f8 = mybir.dt.np(F8)
    T = lambda a: np.ascontiguousarray(np.asarray(a, dtype=f).T)
    c8 = lambda a: np.clip(np.asarray(a, dtype=f), -240, 240).astype(f8)
    sig = lambda a: 1.0 / (1.0 + np.exp(-a))

    Wq_, Wk_, Wv_, Wo_ = (np.asarray(w, f) for w in (Wq, Wk, Wv, Wo))
    wvo = T(Wv_) @ T(Wo_)                    # (Wo Wv)^T
    A = Wq_.T @ Wk_                          # [f, g]
    vq = (np.asarray(bq, f) @ Wk_) * QSC     # [768]

    smf = np.zeros((128, SMF_N), f)
    smf[:, SMF_ID:SMF_ID + 128] = np.eye(128, dtype=f)
    smf[:, SMF_NEG1] = -1.0
    smf[:, SMF_BC1:SMF_BC1 + 3] = np.asarray(bc1, f).reshape(3, 128).T
    bc2p = np.zeros(256, f); bc2p[:192] = np.asarray(bc2, f)
    smf[:, SMF_BC2:SMF_BC2 + 2] = bc2p.reshape(2, 128).T
    smf[:, SMF_BF1:SMF_BF1 + 6] = np.asarray(bf1, f).reshape(HC, 128).T
    smf[:, SMF_VQ:SMF_VQ + 6] = vq.reshape(HC, 128).T
    smf[:, SMF_NBC3] = -float(np.asarray(bc3, f).reshape(-1)[0])
    smf[:, SMF_NBF2] = -float(np.asarray(bf2, f).reshape(-1)[0])

    # pw tables: pos_w / (sqrt(H) * QSC); first/last mask sequence edges
    p_ = np.arange(128)[:, None]; c_ = np.arange(WIN)[None, :]
    pwm = (np.exp(-np.abs(128 + p_ - c_) / 2.0) / (math.sqrt(H) * QSC)).astype(f)

    wc2p = np.zeros((512, 256), f); wc2p[:384, :192] = T(Wc2)
    wc3p = np.zeros((256,), f); wc3p[:192] = np.asarray(Wc3, f).reshape(-1)
    f8pk = (wc3p.reshape(2, 128).T) * WC3_SC

    wf1p = (T(Wf1) / S).astype(bf)                       # [3, 768]
    bvoo = (np.asarray(bv, f) @ Wo_.T + np.asarray(bo, f)).reshape(H)

    # per-batch task scalar
    task_table = np.asarray(task_table, f); task_id = np.asarray(task_id)
    te = task_table[task_id.astype(np.int64)]            # [B, 64]
    t1 = np.maximum(te @ np.asarray(Wt1, f).T + np.asarray(bt1, f), 0.0)
    tw = sig(t1 @ np.asarray(Wt2, f).T + np.asarray(bt2, f))
    ts_all = tw.mean(-1)                                 # [B]

    shared = {
        "a8": c8(A * A_SC), "wvo8": c8(wvo * WVO_SC),
        "wc18": c8(T(Wc1) * WC1_SC), "wc28": c8(wc2p * WC2_SC),
        "f8p": c8(f8pk), "wf1p": wf1p,
    }

    x = np.asarray(x, f); pos_emb = np.asarray(pos_emb, f)
    in_maps = []
    vswo_all = []
    for core in range(8):
        b, half = core // 2, core % 2
        xp_full = x[b] + pos_emb[0]                      # [S, H]
        g0 = 1024 * half - 128
        xp = np.zeros((NB * 128, H), f)
        lo, hi = max(0, g0), min(S, g0 + NB * 128)
        xp[lo - g0:hi - g0] = xp_full[lo:hi]
        # feature-major fp8: [128, HC, NB*128]
        xpT = xp.T.reshape(HC, 128, NB * 128).transpose(1, 0, 2)
        xpT8 = c8(xpT.reshape(128, HC * NB * 128))

        pwf = pwm.copy(); pwl = pwm.copy()
        if half == 0:
            pwf[:, :128] = 0
        if half == 1:
            pwl[:, 256:] = 0
        bfpk = np.zeros((128, NBF), bf)
        bfpk[:, BFP_PW:BFP_PW + WIN] = pwf.astype(bf)
        bfpk[:, BFP_PW + WIN:BFP_PW + 2 * WIN] = pwm.astype(bf)
        bfpk[:, BFP_PW + 2 * WIN:BFP_PW + 3 * WIN] = pwl.astype(bf)
        bfpk[:, BFP_WF2:BFP_WF2 + 6] = \
            np.asarray(Wf2, f).reshape(HC, 128).T.astype(bf)

        smfc = smf.copy()
        ts = float(ts_all[b])
        smfc[:, SMF_TS1] = 1.0 - 0.5 * ts
        smfc[:, SMF_TS] = ts

        m = dict(shared)
        m.update({"xpT8": xpT8, "bfp": bfpk, "smf": smfc})
        in_maps.append(m)
        vswo_all.append((xp_full.sum(0) @ wvo).astype(f))
    return in_maps, bvoo, vswo_all


class _Runner:
    """Compile the SPMD graph once and keep a reusable jitted callable."""

    def __init__(self, nc, n_cores=8):
        import jax
        from jax.sharding import Mesh, PartitionSpec
        from jax.experimental.shard_map import shard_map
        from concourse import bass2jax, mybir as _mb
        bass2jax.install_neuronx_cc_hook()
        self.nc = nc
        partition_name = (nc.partition_id_tensor.name
                          if nc.partition_id_tensor else None)
        in_names, out_names, out_avals, zero_outs = [], [], [], []
        for alloc in nc.m.functions[0].allocations:
            if not isinstance(alloc, _mb.MemoryLocationSet):
                continue
            name = alloc.memorylocations[0].name
            if alloc.kind == "ExternalInput":
                if name != partition_name:
                    in_names.append(name)
            elif alloc.kind == "ExternalOutput":
                shape = tuple(alloc.tensor_shape)
                dtype = _mb.dt.np(alloc.dtype)
                out_names.append(name)
                out_avals.append(jax.core.ShapedArray(shape, dtype))
                zero_outs.append(np.zeros(shape, dtype))
        self.in_names = list(in_names)
        self.out_names = out_names
        self.out_avals = out_avals
        self.zero_outs = zero_outs
        self.n_cores = n_cores
        n_params = len(self.in_names)
        all_in = list(self.in_names) + list(out_names)
        if partition_name is not None:
            all_in.append(partition_name)

        def _body(*args):
            operands = list(args)
            if partition_name is not None:
                operands.append(bass2jax.partition_id_tensor())
            outs = bass2jax._bass_exec_p.bind(
                *operands,
                out_avals=tuple(out_avals),
                in_names=tuple(all_in),
                out_names=tuple(out_names),
                lowering_input_output_aliases=(),
                sim_require_finite=True,
                sim_require_nnan=True,
                nc=nc,
            )
            return tuple(outs)

        devices = jax.devices()[:n_cores]
        mesh = Mesh(np.asarray(devices), ("core",))
        n_outs = len(out_names)
        in_specs = (PartitionSpec("core"),) * (n_params + n_outs)
        out_specs = (PartitionSpec("core"),) * n_outs
        self.fn = jax.jit(
            shard_map(_body, mesh=mesh, in_specs=in_specs,
                      out_specs=out_specs, check_rep=False),
            keep_unused=True)

    def concat_inputs(self, in_maps):
        return [np.concatenate([np.asarray(in_maps[c][k])
                                for c in range(self.n_cores)], axis=0)
                for k in self.in_names]

    def zeros(self):
        return [np.zeros((self.n_cores * z.shape[0],) + z.shape[1:], z.dtype)
                for z in self.zero_outs]

    def __call__(self, concat_in, zeros=None):
        import jax
        if zeros is None:
            zeros = self.zeros()
        outs = jax.block_until_ready(self.fn(*concat_in, *zeros))
        return outs


def get_runner(dbg=False):
    if "runner" not in _cache:
        _cache["runner"] = _Runner(build_kernel())
    return _cache["runner"]


def kernel(**inputs):
    rn = get_runner()
    in_maps, bvoo, vswo_all = prep_inputs(**inputs)
    outs = rn(rn.concat_inputs(in_maps))
    yc = np.asarray(outs[rn.out_names.index("y")]).reshape(8, 1024, H)
    zc = np.asarray(outs[rn.out_names.index("zr8")]).reshape(8, 128, 8)
    out = np.zeros((B, S, H), np.float32)
    for core in range(8):
        b, half = core // 2, core % 2
        zr = zc[core].T.reshape(1024, 1)
        out[b, 1024 * half:1024 * (half + 1)] = \
            yc[core].astype(np.float32) + zr * vswo_all[core] + bvoo
    return out
